# revision 15
# baseline (speedup 1.0000x reference)
"""H2GCN forward pass on 8 Trainium2 NeuronCores (Bass/Tile SPMD kernel), v3.

Strategy (1D row-parallel SpMM + pre-projected conv2):
  - Nodes are sharded across 8 cores (1024 rows each). Each core receives the
    column-slice adjT = adj[rows, :].T (i.e. [8192, 1024]) of both adjacency
    matrices in fp16 - exactly the rhs layout the tensor engine wants.
  - The feature embed is *replicated*: every core computes the full h for all
    8192 nodes directly in node-major layout instead of all-gathering it.
  - conv1 computes zT = [A@h; A2@h].T feature-major, RAW (un-normalized).
  - BatchNorm is *algebraically absorbed* into the final projection:
        z_n = z*c + d  with c = gamma*rsqrt(var+eps), d = beta - mean*c
    After the stats AllReduce, zT is scaled by c IN PLACE; the d-part flows
    through rank-1 corrections (s_j = W_j@d times exact host rowsums).
  - KEY v2 change: conv2's output u = [A@z_n; A2@z_n] is only ever consumed
    by the final projection onto O=64 dims, and (A@z_n)@W1.T = A@(z_n@W1.T).
    So we pre-project: p1 = (c*z)@W1.T, p2 = (c*z)@W2.T [n, 64], and SpMM
    the tiny p instead of the 512-wide z -> 8x fewer conv2 FLOPs; the
    AllGather shrinks from 2x[8192,256]f16 to 1x[8192,128]f16 (chunked in 2
    so conv2' starts on the first half).
  - conv2's SpMM uses the *binary* decomposition adj = dis_i * B * dis_j with
    B in fp8 (0/1 exact, half the HBM traffic of fp16); the dis scalings are
    applied to p (per-partition scalars pre-gather) and to the output columns
    (host-shipped broadcast rows). The matmul mixes fp16 lhsT x fp8 rhs.
  - v3 scheduling: minimal pre-embed DMA; per-half BN stats pre-issued on
    scalar+vector so only ~2us trails conv1; sqrt activation table preloaded;
    the local hT embed, s-vector/rank-1 prep and the h/z-block final matmuls
    fill the AllReduce/AllGather latency windows.
"""

import numpy as np
import ml_dtypes

import concourse.bass as bass
import concourse.mybir as mybir
import concourse.tile as tile
from concourse import bacc
from concourse.bass_utils import run_bass_kernel_spmd
from concourse.masks import make_identity

P = 128
NCORES = 8
BN_EPS = 1e-5

F16 = mybir.dt.float16
F32 = mybir.dt.float32
F8 = mybir.dt.float8e4

B_FP8 = True

FULL_CFG = dict(NT=8192, R=1024)
IN_CH = 512   # input features
H = 256       # hidden
H2 = 512      # 2*H (BN width)
O = 64        # output features
F = 7 * H     # 1792, JK concat width


def _nchunks(R):
    """Split the per-core node free-dim R into <=512 chunks (PSUM bank width)."""
    out = []
    s = 0
    while s < R:
        w = min(512, R - s)
        out.append((s, w))
        s += w
    return out


def build_program(NT, R):
    """Build the SPMD Bass program. NT = total nodes, R = rows per core."""
    KT = NT // P           # node k-tiles (contraction tiles)
    RT = R // P            # per-core node tiles (free-dim tiles / transposes)
    NCH = _nchunks(R)
    NC2 = len(NCH)
    HM = H // P            # 2  (hidden chunks)
    H2M = H2 // P          # 4
    FM = F // P            # 14
    INK = IN_CH // P       # 4
    BDT = F8 if B_FP8 else F16
    NGC = 2                # p-AllGather chunks
    RTC = RT // NGC

    nc = bacc.Bacc("TRN2", target_bir_lowering=False, debug=False,
                   num_devices=NCORES)

    # --- I/O -------------------------------------------------------------
    xTf = nc.dram_tensor("xTf", [IN_CH, NT], F16, kind="ExternalInput")
    xT = nc.dram_tensor("xT", [IN_CH, R], F16, kind="ExternalInput")
    adjT = nc.dram_tensor("adjT", [NT, R], F16, kind="ExternalInput")
    adjT2 = nc.dram_tensor("adjT2", [NT, R], F16, kind="ExternalInput")
    Bp1 = nc.dram_tensor("Bp1", [NT, R], BDT, kind="ExternalInput")
    Bp2 = nc.dram_tensor("Bp2", [NT, R], BDT, kind="ExternalInput")
    wTe = nc.dram_tensor("wTe", [IN_CH, H], F16, kind="ExternalInput")
    be = nc.dram_tensor("be", [P, HM], F32, kind="ExternalInput")
    bebc = nc.dram_tensor("bebc", [P, H], F32, kind="ExternalInput")
    wTf = nc.dram_tensor("wTf", [F, O], F16, kind="ExternalInput")
    bff = nc.dram_tensor("bff", [O, 1], F32, kind="ExternalInput")
    gam = nc.dram_tensor("gam", [P, H2M], F32, kind="ExternalInput")
    bet = nc.dram_tensor("bet", [P, H2M], F32, kind="ExternalInput")
    rsA = nc.dram_tensor("rsA", [O, R], F32, kind="ExternalInput")
    rsA2 = nc.dram_tensor("rsA2", [O, R], F32, kind="ExternalInput")
    dis1L = nc.dram_tensor("dis1L", [P, RT], F32, kind="ExternalInput")
    dis2L = nc.dram_tensor("dis2L", [P, RT], F32, kind="ExternalInput")
    disr1 = nc.dram_tensor("disr1", [O, R], F32, kind="ExternalInput")
    disr2 = nc.dram_tensor("disr2", [O, R], F32, kind="ExternalInput")
    out = nc.dram_tensor("out", [R, O], F32, kind="ExternalOutput")

    rg = [list(range(NCORES))]

    with tile.TileContext(nc) as tc:
        with (
            tc.tile_pool(name="const", bufs=1) as const,
            tc.tile_pool(name="feat", bufs=1) as feat,
            tc.tile_pool(name="tmp", bufs=2) as tmp,
            tc.tile_pool(name="stream", bufs=8) as stream,
            tc.tile_pool(name="ps", bufs=1, space="PSUM") as ps,
            tc.tile_pool(name="dram", bufs=1, space="DRAM") as dram,
        ):
            # --- minimal embed-critical DMA first -----------------------
            wTe_sb = const.tile([P, INK, H], F16, name="wTe_sb")
            nc.sync.dma_start(wTe_sb[:], wTe.ap().rearrange("(k p) m -> p k m", p=P))
            bebc_sb = const.tile([P, H], F32, name="bebc_sb")
            nc.sync.dma_start(bebc_sb[:], bebc.ap())

            # full x.T in node-chunk groups; first groups small so the embed
            # starts as early as possible
            xTf_t = xTf.ap().rearrange("(k p) n -> p k n", p=P)
            xTf_sb = feat.tile([P, INK, NT], F16, name="xTf_sb", tag="kxnB")
            xgrps = [(0, 512), (512, 512)]
            g = 1024
            while g < NT:
                xgrps.append((g, 1024))
                g += 1024
            for gs, gw in xgrps:
                nc.sync.dma_start(xTf_sb[:, :, gs:gs + gw], xTf_t[:, :, gs:gs + gw])

            # --- phase B1: replicated full embed, node-major ------------
            hfull_sb = feat.tile([P, KT, H], F16, name="hfull_sb", tag="kxnA")
            for k in range(KT):
                hps = ps.tile([P, H], F32, name=f"hps_{k}", tag=f"acc{k % 8}")
                for t in range(INK):
                    nc.tensor.matmul(
                        hps[:],
                        lhsT=xTf_sb[:, t, k * P:(k + 1) * P],
                        rhs=wTe_sb[:, t, :],
                        start=(t == 0), stop=(t == INK - 1),
                    )
                nc.vector.tensor_tensor(
                    out=hfull_sb[:, k, :], in0=hps[:], in1=bebc_sb[:],
                    op=mybir.AluOpType.add)
                nc.scalar.activation(
                    hfull_sb[:, k, :], hfull_sb[:, k, :],
                    mybir.ActivationFunctionType.Relu)

            # --- bulk constants (off the embed critical path) -----------
            xT_sb = const.tile([P, INK, R], F16, name="xT_sb")
            nc.sync.dma_start(xT_sb[:], xT.ap().rearrange("(k p) n -> p k n", p=P))
            be_sb = const.tile([P, HM], F32, name="be_sb")
            nc.sync.dma_start(be_sb[:], be.ap())
            id16 = const.tile([P, P], F16, name="id16")
            make_identity(nc, id16)
            id32 = const.tile([P, P], F32, name="id32")
            make_identity(nc, id32)
            gam_sb = const.tile([P, H2M], F32, name="gam_sb")
            nc.sync.dma_start(gam_sb[:], gam.ap())
            bet_sb = const.tile([P, H2M], F32, name="bet_sb")
            nc.sync.dma_start(bet_sb[:], bet.ap())
            wTf_sb = const.tile([P, FM, O], F16, name="wTf_sb")
            nc.sync.dma_start(wTf_sb[:], wTf.ap().rearrange("(k p) m -> p k m", p=P))
            bff_sb = const.tile([O, 1], F32, name="bff_sb")
            nc.sync.dma_start(bff_sb[:], bff.ap())
            rsA_sb = const.tile([O, R], F32, name="rsA_sb")
            nc.sync.dma_start(rsA_sb[:], rsA.ap())
            rsA2_sb = const.tile([O, R], F32, name="rsA2_sb")
            nc.sync.dma_start(rsA2_sb[:], rsA2.ap())
            disr1_sb = const.tile([O, R], F32, name="disr1_sb")
            nc.sync.dma_start(disr1_sb[:], disr1.ap())
            disr2_sb = const.tile([O, R], F32, name="disr2_sb")
            nc.sync.dma_start(disr2_sb[:], disr2.ap())
            dis1L_sb = const.tile([P, RT], F32, name="dis1L_sb")
            nc.sync.dma_start(dis1L_sb[:], dis1L.ap())
            dis2L_sb = const.tile([P, RT], F32, name="dis2L_sb")
            nc.sync.dma_start(dis2L_sb[:], dis2L.ap())

            # --- phase D: conv1, zT = [A@h; A2@h].T (raw), one adjacency
            # half at a time. After each half: partial BN stats + its own
            # tiny AllReduce. AR0 runs hidden under conv1-half1 (and warms
            # up / skew-syncs the CC engine for AR1).
            zT_sb = feat.tile([P, H2M, R], F16, name="zT_sb")
            eps_sb = tmp.tile([P, 1], F32, name="eps_sb", bufs=1)
            nc.vector.memset(eps_sb[:], BN_EPS)
            c_t = tmp.tile([P, H2M], F32, name="c_t", bufs=1)
            d_t = tmp.tile([P, H2M], F32, name="d_t", bufs=1)
            hT_sb = feat.tile([P, HM, R], F16, name="hT_sb")
            stat_g = {}

            def bn_chain(half):
                """c, d for this half's two feature chunks from global stats."""
                sg = stat_g[half]
                f0 = half * HM
                cmean = tmp.tile([P, HM], F32, name=f"cmean_{half}", bufs=1)
                nc.scalar.mul(cmean[:], sg[:, 0:HM], 1.0 / NT)
                cvar = tmp.tile([P, HM], F32, name=f"cvar_{half}", bufs=1)
                nc.scalar.mul(cvar[:], sg[:, HM:2 * HM], 1.0 / NT)
                msq = tmp.tile([P, HM], F32, name=f"msq_{half}", bufs=1)
                nc.vector.tensor_mul(out=msq[:], in0=cmean[:], in1=cmean[:])
                nc.vector.tensor_tensor(
                    out=cvar[:], in0=cvar[:], in1=msq[:],
                    op=mybir.AluOpType.subtract)
                cstd = tmp.tile([P, HM], F32, name=f"cstd_{half}", bufs=1)
                nc.scalar.activation(
                    cstd[:], cvar[:], mybir.ActivationFunctionType.Sqrt,
                    bias=eps_sb[:])
                crstd = tmp.tile([P, HM], F32, name=f"crstd_{half}", bufs=1)
                nc.vector.reciprocal(crstd[:], cstd[:])
                nc.vector.tensor_mul(
                    out=c_t[:, f0:f0 + HM], in0=crstd[:],
                    in1=gam_sb[:, f0:f0 + HM])
                nc.vector.tensor_mul(
                    out=d_t[:, f0:f0 + HM], in0=cmean[:],
                    in1=c_t[:, f0:f0 + HM])
                nc.vector.tensor_tensor(
                    out=d_t[:, f0:f0 + HM], in0=bet_sb[:, f0:f0 + HM],
                    in1=d_t[:, f0:f0 + HM], op=mybir.AluOpType.subtract)
                # scale this half's zT chunks by c in place
                for m in range(HM):
                    f = f0 + m
                    nc.vector.tensor_scalar_mul(
                        zT_sb[:, f, :], zT_sb[:, f, :], c_t[:, f:f + 1])

            for half, src in ((0, adjT), (1, adjT2)):
                zps = {}
                for m in range(HM):
                    for ci in range(NC2):
                        zps[(m, ci)] = ps.tile(
                            [P, 512], F32, name=f"zps_{half}_{m}_{ci}",
                            tag=f"acc{(half * 4 + m * NC2 + ci) % 8}")
                for k in range(KT):
                    at = stream.tile([P, R], F16, name=f"c1_{half}_{k}", tag="adj", bufs=6)
                    nc.sync.dma_start(at[:], src[k * P:(k + 1) * P, :])
                    for m in range(HM):
                        for ci, (cs, cw) in enumerate(NCH):
                            nc.tensor.matmul(
                                zps[(m, ci)][:, :cw],
                                lhsT=hfull_sb[:, k, m * P:(m + 1) * P],
                                rhs=at[:, cs:cs + cw],
                                start=(k == 0), stop=(k == KT - 1),
                            )
                for m in range(HM):
                    for ci, (cs, cw) in enumerate(NCH):
                        nc.vector.tensor_copy(
                            out=zT_sb[:, half * HM + m, cs:cs + cw],
                            in_=zps[(m, ci)][:, :cw])
                # BN partial stats for this half's features: sums on vector,
                # square-sums on scalar (concurrent engines), then AllReduce
                stat_sb = tmp.tile([P, 2 * HM], F32, name=f"stat_{half}", bufs=1)
                for m in range(HM):
                    f = half * HM + m
                    nc.vector.tensor_reduce(
                        out=stat_sb[:, m:m + 1], in_=zT_sb[:, f, :],
                        axis=mybir.AxisListType.X, op=mybir.AluOpType.add)
                    sq2 = tmp.tile([P, R], F16, name="sq2", tag="sq", bufs=2)
                    nc.scalar.activation(
                        sq2[:], zT_sb[:, f, :],
                        mybir.ActivationFunctionType.Square,
                        accum_out=stat_sb[:, HM + m:HM + m + 1])
                ar_in = dram.tile([P, 2 * HM], F32, name=f"ar_in_{half}")
                nc.gpsimd.dma_start(ar_in[:], stat_sb[:])
                ar_out = dram.tile([P, 2 * HM], F32, name=f"ar_out_{half}")
                nc.gpsimd.collective_compute(
                    "AllReduce", mybir.AluOpType.add, replica_groups=rg,
                    ins=[ar_in.opt()], outs=[ar_out.opt()],
                )
                sg = tmp.tile([P, 2 * HM], F32, name=f"stat_g_{half}", bufs=1)
                nc.gpsimd.dma_start(sg[:], ar_out[:])
                stat_g[half] = sg
                if half == 0:
                    # hidden under conv1-half1: c/d for half0 features +
                    # partial p-projection over those features
                    bn_chain(0)

            # --- post-conv1: fill the AR1 latency window ----------------
            # local hT embed (JK h-block), h-block final matmuls, partial
            # p-projections over half0 z-features, sqrt-table re-warm
            fin = {}
            pps = {}
            for ci in range(NC2):
                fin[ci] = ps.tile([O, 512], F32, name=f"fin_{ci}", tag=f"acc{ci}")
            for j in range(2):
                for ci in range(NC2):
                    pps[(j, ci)] = ps.tile([O, 512], F32, name=f"pps_{j}_{ci}",
                                           tag=f"acc{2 + j * NC2 + ci}")
            for m in range(HM):
                for ci, (cs, cw) in enumerate(NCH):
                    eps_t = ps.tile([P, 512], F32, name=f"eps_{m}_{ci}",
                                    tag=f"acc{6 + (m * NC2 + ci) % 2}")
                    for t in range(INK):
                        nc.tensor.matmul(
                            eps_t[:, :cw],
                            lhsT=wTe_sb[:, t, m * P:(m + 1) * P],
                            rhs=xT_sb[:, t, cs:cs + cw],
                            start=(t == 0), stop=(t == INK - 1),
                        )
                    nc.scalar.activation(
                        hT_sb[:, m, cs:cs + cw], eps_t[:, :cw],
                        mybir.ActivationFunctionType.Relu,
                        bias=be_sb[:, m:m + 1],
                    )
            for ci, (cs, cw) in enumerate(NCH):
                for t in range(HM):
                    nc.tensor.matmul(
                        fin[ci][:, :cw], lhsT=wTf_sb[:, t, :],
                        rhs=hT_sb[:, t, cs:cs + cw],
                        start=(t == 0), stop=False)
            for j, base in enumerate((HM + H2M, HM + 2 * H2M)):
                for ci, (cs, cw) in enumerate(NCH):
                    for t in range(HM):
                        nc.tensor.matmul(
                            pps[(j, ci)][:, :cw], lhsT=wTf_sb[:, base + t, :],
                            rhs=zT_sb[:, t, cs:cs + cw],
                            start=(t == 0), stop=False)
            warm = tmp.tile([P, 1], F32, name="warm", bufs=1)
            nc.scalar.activation(
                warm[:], eps_sb[:], mybir.ActivationFunctionType.Sqrt,
                bias=eps_sb[:])

            # AR1 lands: finish BN for half1 features, then the rest of the
            # p-projection; per-ci ordering so AG chunk 0 launches early
            bn_chain(1)
            pT_sb = tmp.tile([P, R], F16, name="pT_sb", bufs=1)
            pcat_nm = tmp.tile([P, RT, P], F16, name="pcat_nm", bufs=1)
            pg_sb = feat.tile([P, NCORES, RT, P], F16, name="pg_sb")
            pg_chunks = []
            for gc in range(NGC):
                cs, cw = NCH[gc]
                for j, base in enumerate((HM + H2M, HM + 2 * H2M)):
                    for t in range(HM, H2M):
                        nc.tensor.matmul(
                            pps[(j, gc)][:, :cw], lhsT=wTf_sb[:, base + t, :],
                            rhs=zT_sb[:, t, cs:cs + cw],
                            start=False, stop=(t == H2M - 1))
                    nc.vector.tensor_copy(
                        out=pT_sb[j * O:(j + 1) * O, cs:cs + cw],
                        in_=pps[(j, gc)][:, :cw])
                for nt in range(gc * RTC, (gc + 1) * RTC):
                    tps = ps.tile([P, P], F16, name=f"ptp_{nt}",
                                  tag=f"acc{6 + nt % 2}")
                    nc.tensor.transpose(
                        tps[:], pT_sb[:, nt * P:(nt + 1) * P], id16[:])
                    nc.vector.tensor_scalar_mul(
                        pcat_nm[:, nt, 0:O], tps[:, 0:O], dis1L_sb[:, nt:nt + 1])
                    nc.vector.tensor_scalar_mul(
                        pcat_nm[:, nt, O:P], tps[:, O:P], dis2L_sb[:, nt:nt + 1])
                pg_in = dram.tile([RTC * P, P], F16, name=f"pg_in_{gc}")
                nc.gpsimd.dma_start(
                    pg_in.rearrange("(nt p) f -> p nt f", p=P),
                    pcat_nm[:, gc * RTC:(gc + 1) * RTC, :])
                pg_o = dram.tile([NCORES, RTC * P, P], F16, name=f"pg_out_{gc}",
                                 addr_space="Shared")
                nc.gpsimd.collective_compute(
                    "AllGather", mybir.AluOpType.bypass, replica_groups=rg,
                    ins=[pg_in.opt()], outs=[pg_o.opt()],
                )
                pg_chunks.append(pg_o)
                for cr in range(NCORES):
                    nc.gpsimd.dma_start(
                        pg_sb[:, cr, gc * RTC:(gc + 1) * RTC, :],
                        pg_o[cr].rearrange("(nt p) f -> p nt f", p=P))

            # fill the AllGather window: s vectors, rank-1 prep, z-block
            # final matmuls (zT now c-scaled, wTf stays UNSCALED)
            d16 = tmp.tile([P, H2M], F16, name="d16", bufs=1)
            nc.vector.tensor_copy(out=d16[:], in_=d_t[:])
            s_cols = tmp.tile([O, 3], F32, name="s_cols", bufs=1)
            for j, base in enumerate((HM, HM + H2M, HM + 2 * H2M)):
                sps = ps.tile([O, 1], F32, name=f"sps_{j}", tag="acc6")
                for t in range(H2M):
                    nc.tensor.matmul(
                        sps[:], lhsT=wTf_sb[:, base + t, :],
                        rhs=d16[:, t:t + 1],
                        start=(t == 0), stop=(t == H2M - 1))
                nc.vector.tensor_copy(out=s_cols[:, j:j + 1], in_=sps[:])
            s0b = tmp.tile([O, 1], F32, name="s0b", bufs=1)
            nc.vector.tensor_add(out=s0b[:], in0=s_cols[:, 0:1], in1=bff_sb[:])
            # rkc = s0 + bias + s1*rsA + s2*rsA2  (shared across both ci)
            rkc = tmp.tile([O, R], F32, name="rkc", bufs=1)
            nc.vector.tensor_scalar_mul(rkc[:], rsA_sb[:], s_cols[:, 1:2])
            rk2 = tmp.tile([O, R], F32, name="rk2", bufs=1)
            nc.vector.tensor_scalar_mul(rk2[:], rsA2_sb[:], s_cols[:, 2:3])
            nc.vector.tensor_add(out=rkc[:], in0=rkc[:], in1=rk2[:])
            nc.vector.tensor_scalar_add(rkc[:], rkc[:], s0b[:])
            # z-block final matmuls (continue the fin PSUM groups)
            for ci, (cs, cw) in enumerate(NCH):
                for t in range(H2M):
                    nc.tensor.matmul(
                        fin[ci][:, :cw], lhsT=wTf_sb[:, HM + t, :],
                        rhs=zT_sb[:, t, cs:cs + cw],
                        start=False, stop=(t == H2M - 1))

            # --- phase H: conv2' binary SpMM (k ordered by gather chunk) -
            q1 = {}
            q2 = {}
            for ci in range(NC2):
                q1[ci] = ps.tile([O, 512], F32, name=f"q1_{ci}", tag=f"acc{2 + ci}")
                q2[ci] = ps.tile([O, 512], F32, name=f"q2_{ci}", tag=f"acc{4 + ci}")
            for gc in range(NGC):
                for cr in range(NCORES):
                    for nt in range(gc * RTC, (gc + 1) * RTC):
                        k = cr * RT + nt
                        first = (gc == 0 and cr == 0 and nt == 0)
                        last = (gc == NGC - 1 and cr == NCORES - 1
                                and nt == (gc + 1) * RTC - 1)
                        bt1 = stream.tile([P, R], BDT, name=f"b1_{k}",
                                          tag="adj8", bufs=14)
                        nc.sync.dma_start(bt1[:], Bp1[k * P:(k + 1) * P, :])
                        bt2 = stream.tile([P, R], BDT, name=f"b2_{k}",
                                          tag="adj8", bufs=14)
                        nc.sync.dma_start(bt2[:], Bp2[k * P:(k + 1) * P, :])
                        for ci, (cs, cw) in enumerate(NCH):
                            nc.tensor.matmul(
                                q1[ci][:, :cw], lhsT=pg_sb[:, cr, nt, 0:O],
                                rhs=bt1[:, cs:cs + cw],
                                start=first, stop=last)
                        for ci, (cs, cw) in enumerate(NCH):
                            nc.tensor.matmul(
                                q2[ci][:, :cw], lhsT=pg_sb[:, cr, nt, O:P],
                                rhs=bt2[:, cs:cs + cw],
                                start=first, stop=last)

            # combine: out = fin + rkc + dis1_i*q1 + dis2_i*q2; then per-ci
            # transpose [O, .] -> node-major and write out incrementally
            outsb = tmp.tile([O, R], F32, name="outsb", bufs=1)
            o_nm = tmp.tile([P, RT, O], F32, name="o_nm", bufs=1)
            out_t = out.ap().rearrange("(nt p) o -> p nt o", p=P)
            for ci, (cs, cw) in enumerate(NCH):
                nc.vector.tensor_add(
                    out=outsb[:, cs:cs + cw], in0=fin[ci][:, :cw],
                    in1=rkc[:, cs:cs + cw])
                t1 = tmp.tile([O, 512], F32, name=f"t1_{ci}", tag="cmb", bufs=2)
                nc.vector.tensor_mul(
                    out=t1[:, :cw], in0=q1[ci][:, :cw],
                    in1=disr1_sb[:, cs:cs + cw])
                nc.vector.tensor_add(
                    out=outsb[:, cs:cs + cw], in0=outsb[:, cs:cs + cw],
                    in1=t1[:, :cw])
                t2 = tmp.tile([O, 512], F32, name=f"t2_{ci}", tag="cmb", bufs=2)
                nc.vector.tensor_mul(
                    out=t2[:, :cw], in0=q2[ci][:, :cw],
                    in1=disr2_sb[:, cs:cs + cw])
                nc.vector.tensor_add(
                    out=outsb[:, cs:cs + cw], in0=outsb[:, cs:cs + cw],
                    in1=t2[:, :cw])
                for nt in range(ci * RT // NC2, (ci + 1) * RT // NC2):
                    tps32 = ps.tile([P, O], F32, name=f"otp_{nt}",
                                    tag=f"acc{6 + nt % 2}")
                    nc.tensor.transpose(
                        tps32[:], outsb[:, nt * P:(nt + 1) * P], id32[:O, :O])
                    nc.any.tensor_copy(out=o_nm[:, nt, :], in_=tps32[:])
                nc.sync.dma_start(
                    out_t[:, ci * RT // NC2:(ci + 1) * RT // NC2, :],
                    o_nm[:, ci * RT // NC2:(ci + 1) * RT // NC2, :])

    nc.compile()
    return nc


_PROGRAM_CACHE = {}


def _get_program(NT, R):
    key = (NT, R)
    if key not in _PROGRAM_CACHE:
        _PROGRAM_CACHE[key] = build_program(NT, R)
    return _PROGRAM_CACHE[key]


def make_in_maps(inputs, NT, R):
    """Shard full inputs into per-core input maps (host-side, numpy)."""
    RT = R // P
    x = np.asarray(inputs["x"], np.float32)
    adj = np.asarray(inputs["adj_t"], np.float32)
    adj2 = np.asarray(inputs["adj_t2"], np.float32)
    we = np.asarray(inputs["w_embed"], np.float32)
    be = np.asarray(inputs["b_embed"], np.float32)
    gam = np.asarray(inputs["bn_gamma"], np.float32)
    bet = np.asarray(inputs["bn_beta"], np.float32)
    wf = np.asarray(inputs["w_fin"], np.float32)
    bf = np.asarray(inputs["b_fin"], np.float32)

    H2M = H2 // P
    xTf_h = np.ascontiguousarray(x.T).astype(np.float16)
    wTe_h = np.ascontiguousarray(we.T).astype(np.float16)
    be_h = np.ascontiguousarray(be.reshape(H // P, P).T).astype(np.float32)
    bebc_h = np.ascontiguousarray(
        np.broadcast_to(be[None, :], (P, H))).astype(np.float32)
    wTf_h = np.ascontiguousarray(wf.T).astype(np.float16)
    bff_h = np.ascontiguousarray(bf[:, None]).astype(np.float32)
    gam_h = np.ascontiguousarray(gam.reshape(H2M, P).T).astype(np.float32)
    bet_h = np.ascontiguousarray(bet.reshape(H2M, P).T).astype(np.float32)

    # binary decomposition of the normalized adjacencies
    B1 = adj > 0
    B2 = adj2 > 0
    d1 = B1.sum(1).astype(np.float32)
    d2 = B2.sum(1).astype(np.float32)
    dis1 = np.where(d1 > 0, 1.0 / np.sqrt(np.maximum(d1, 1e-12)), 0.0
                    ).astype(np.float32)
    dis2 = np.where(d2 > 0, 1.0 / np.sqrt(np.maximum(d2, 1e-12)), 0.0
                    ).astype(np.float32)
    bdt = ml_dtypes.float8_e4m3 if B_FP8 else np.float16

    in_maps = []
    for r in range(NCORES):
        rows = slice(r * R, (r + 1) * R)
        rsA_h = np.ascontiguousarray(
            np.broadcast_to(adj[rows].sum(1)[None, :], (O, R))).astype(np.float32)
        rsA2_h = np.ascontiguousarray(
            np.broadcast_to(adj2[rows].sum(1)[None, :], (O, R))).astype(np.float32)
        disr1_h = np.ascontiguousarray(
            np.broadcast_to(dis1[rows][None, :], (O, R))).astype(np.float32)
        disr2_h = np.ascontiguousarray(
            np.broadcast_to(dis2[rows][None, :], (O, R))).astype(np.float32)
        in_maps.append({
            "xTf": xTf_h,
            "xT": np.ascontiguousarray(x[rows].T).astype(np.float16),
            "adjT": np.ascontiguousarray(adj[rows].T).astype(np.float16),
            "adjT2": np.ascontiguousarray(adj2[rows].T).astype(np.float16),
            "Bp1": np.ascontiguousarray(B1[rows].T).astype(bdt),
            "Bp2": np.ascontiguousarray(B2[rows].T).astype(bdt),
            "wTe": wTe_h, "be": be_h, "bebc": bebc_h, "wTf": wTf_h,
            "bff": bff_h, "gam": gam_h, "bet": bet_h,
            "rsA": rsA_h, "rsA2": rsA2_h,
            "dis1L": np.ascontiguousarray(
                dis1[rows].reshape(RT, P).T).astype(np.float32),
            "dis2L": np.ascontiguousarray(
                dis2[rows].reshape(RT, P).T).astype(np.float32),
            "disr1": disr1_h, "disr2": disr2_h,
        })
    return in_maps


def kernel(**inputs):
    NT, R = FULL_CFG["NT"], FULL_CFG["R"]
    nc = _get_program(NT, R)
    in_maps = make_in_maps(inputs, NT, R)
    res = run_bass_kernel_spmd(nc, in_maps, core_ids=list(range(NCORES)))
    out = np.concatenate(
        [res.results[r]["out"] for r in range(NCORES)], axis=0)
    return out.astype(np.float32)


# revision 16
# speedup vs baseline: 1.1303x; 1.1303x over previous
"""H2GCN forward pass on 8 Trainium2 NeuronCores (Bass/Tile SPMD kernel), v3.

Strategy (1D row-parallel SpMM + pre-projected conv2):
  - Nodes are sharded across 8 cores (1024 rows each). Each core receives the
    column-slice adjT = adj[rows, :].T (i.e. [8192, 1024]) of both adjacency
    matrices in fp16 - exactly the rhs layout the tensor engine wants.
  - The feature embed is *replicated*: every core computes the full h for all
    8192 nodes directly in node-major layout instead of all-gathering it.
  - conv1 computes zT = [A@h; A2@h].T feature-major, RAW (un-normalized).
  - BatchNorm is *algebraically absorbed* into the final projection:
        z_n = z*c + d  with c = gamma*rsqrt(var+eps), d = beta - mean*c
    After the stats AllReduce, zT is scaled by c IN PLACE; the d-part flows
    through rank-1 corrections (s_j = W_j@d times exact host rowsums).
  - KEY v2 change: conv2's output u = [A@z_n; A2@z_n] is only ever consumed
    by the final projection onto O=64 dims, and (A@z_n)@W1.T = A@(z_n@W1.T).
    So we pre-project: p1 = (c*z)@W1.T, p2 = (c*z)@W2.T [n, 64], and SpMM
    the tiny p instead of the 512-wide z -> 8x fewer conv2 FLOPs; the
    AllGather shrinks from 2x[8192,256]f16 to 1x[8192,128]f16 (chunked in 2
    so conv2' starts on the first half).
  - conv2's SpMM uses the *binary* decomposition adj = dis_i * B * dis_j with
    B in fp8 (0/1 exact, half the HBM traffic of fp16); the dis scalings are
    applied to p (per-partition scalars pre-gather) and to the output columns
    (host-shipped broadcast rows). The matmul mixes fp16 lhsT x fp8 rhs.
  - v3 scheduling: minimal pre-embed DMA; per-half BN stats pre-issued on
    scalar+vector so only ~2us trails conv1; sqrt activation table preloaded;
    the local hT embed, s-vector/rank-1 prep and the h/z-block final matmuls
    fill the AllReduce/AllGather latency windows.
"""

import numpy as np
import ml_dtypes

import concourse.bass as bass
import concourse.mybir as mybir
import concourse.tile as tile
from concourse import bacc
from concourse.bass_utils import run_bass_kernel_spmd
from concourse.masks import make_identity

P = 128
NCORES = 8
BN_EPS = 1e-5

F16 = mybir.dt.float16
F32 = mybir.dt.float32
F8 = mybir.dt.float8e4

B_FP8 = True

FULL_CFG = dict(NT=8192, R=1024)
IN_CH = 512   # input features
H = 256       # hidden
H2 = 512      # 2*H (BN width)
O = 64        # output features
F = 7 * H     # 1792, JK concat width


def _nchunks(R):
    """Split the per-core node free-dim R into <=512 chunks (PSUM bank width)."""
    out = []
    s = 0
    while s < R:
        w = min(512, R - s)
        out.append((s, w))
        s += w
    return out


def build_program(NT, R):
    """Build the SPMD Bass program. NT = total nodes, R = rows per core."""
    KT = NT // P           # node k-tiles (contraction tiles)
    RT = R // P            # per-core node tiles (free-dim tiles / transposes)
    NCH = _nchunks(R)
    NC2 = len(NCH)
    HM = H // P            # 2  (hidden chunks)
    H2M = H2 // P          # 4
    FM = F // P            # 14
    INK = IN_CH // P       # 4
    BDT = F8 if B_FP8 else F16
    NGC = 2                # p-AllGather chunks
    RTC = RT // NGC

    nc = bacc.Bacc("TRN2", target_bir_lowering=False, debug=False,
                   num_devices=NCORES)

    # --- I/O -------------------------------------------------------------
    xTf = nc.dram_tensor("xTf", [IN_CH, NT], F16, kind="ExternalInput")
    xT = nc.dram_tensor("xT", [IN_CH, R], F16, kind="ExternalInput")
    adjT = nc.dram_tensor("adjT", [NT, R], F16, kind="ExternalInput")
    adjT2 = nc.dram_tensor("adjT2", [NT, R], F16, kind="ExternalInput")
    Bp1 = nc.dram_tensor("Bp1", [NT, R], BDT, kind="ExternalInput")
    Bp2 = nc.dram_tensor("Bp2", [NT, R], BDT, kind="ExternalInput")
    wTe = nc.dram_tensor("wTe", [IN_CH, H], F16, kind="ExternalInput")
    be = nc.dram_tensor("be", [P, HM], F32, kind="ExternalInput")
    bebc = nc.dram_tensor("bebc", [P, H], F32, kind="ExternalInput")
    wTf = nc.dram_tensor("wTf", [F, O], F16, kind="ExternalInput")
    bff = nc.dram_tensor("bff", [O, 1], F32, kind="ExternalInput")
    gam = nc.dram_tensor("gam", [P, H2M], F32, kind="ExternalInput")
    bet = nc.dram_tensor("bet", [P, H2M], F32, kind="ExternalInput")
    rsA = nc.dram_tensor("rsA", [O, R], F32, kind="ExternalInput")
    rsA2 = nc.dram_tensor("rsA2", [O, R], F32, kind="ExternalInput")
    dis1L = nc.dram_tensor("dis1L", [P, RT], F32, kind="ExternalInput")
    dis2L = nc.dram_tensor("dis2L", [P, RT], F32, kind="ExternalInput")
    disr1 = nc.dram_tensor("disr1", [O, R], F32, kind="ExternalInput")
    disr2 = nc.dram_tensor("disr2", [O, R], F32, kind="ExternalInput")
    out = nc.dram_tensor("out", [R, O], F32, kind="ExternalOutput")

    rg = [list(range(NCORES))]

    with tile.TileContext(nc) as tc:
        with (
            tc.tile_pool(name="const", bufs=1) as const,
            tc.tile_pool(name="feat", bufs=1) as feat,
            tc.tile_pool(name="tmp", bufs=2) as tmp,
            tc.tile_pool(name="stream", bufs=8) as stream,
            tc.tile_pool(name="ps", bufs=1, space="PSUM") as ps,
            tc.tile_pool(name="dram", bufs=1, space="DRAM") as dram,
        ):
            # --- minimal embed-critical DMA first -----------------------
            wTe_sb = const.tile([P, INK, H], F16, name="wTe_sb")
            nc.sync.dma_start(wTe_sb[:], wTe.ap().rearrange("(k p) m -> p k m", p=P))
            bebc_sb = const.tile([P, H], F32, name="bebc_sb")
            nc.sync.dma_start(bebc_sb[:], bebc.ap())

            # full x.T in node-chunk groups; first groups small so the embed
            # starts as early as possible
            xTf_t = xTf.ap().rearrange("(k p) n -> p k n", p=P)
            xTf_sb = feat.tile([P, INK, NT], F16, name="xTf_sb", tag="kxnB")
            xgrps = [(0, 512), (512, 512)]
            g = 1024
            while g < NT:
                xgrps.append((g, 1024))
                g += 1024
            for gs, gw in xgrps:
                nc.sync.dma_start(xTf_sb[:, :, gs:gs + gw], xTf_t[:, :, gs:gs + gw])

            # --- phase B1: replicated full embed, node-major ------------
            hfull_sb = feat.tile([P, KT, H], F16, name="hfull_sb", tag="kxnA")
            for k in range(KT):
                hps = ps.tile([P, H], F32, name=f"hps_{k}", tag=f"acc{k % 8}")
                for t in range(INK):
                    nc.tensor.matmul(
                        hps[:],
                        lhsT=xTf_sb[:, t, k * P:(k + 1) * P],
                        rhs=wTe_sb[:, t, :],
                        start=(t == 0), stop=(t == INK - 1),
                    )
                nc.vector.tensor_tensor(
                    out=hfull_sb[:, k, :], in0=hps[:], in1=bebc_sb[:],
                    op=mybir.AluOpType.add)
                nc.scalar.activation(
                    hfull_sb[:, k, :], hfull_sb[:, k, :],
                    mybir.ActivationFunctionType.Relu)

            # --- bulk constants (off the embed critical path) -----------
            xT_sb = const.tile([P, INK, R], F16, name="xT_sb")
            nc.sync.dma_start(xT_sb[:], xT.ap().rearrange("(k p) n -> p k n", p=P))
            be_sb = const.tile([P, HM], F32, name="be_sb")
            nc.sync.dma_start(be_sb[:], be.ap())
            id16 = const.tile([P, P], F16, name="id16")
            make_identity(nc, id16)
            id32 = const.tile([P, P], F32, name="id32")
            make_identity(nc, id32)
            gam_sb = const.tile([P, H2M], F32, name="gam_sb")
            nc.sync.dma_start(gam_sb[:], gam.ap())
            bet_sb = const.tile([P, H2M], F32, name="bet_sb")
            nc.sync.dma_start(bet_sb[:], bet.ap())
            wTf_sb = const.tile([P, FM, O], F16, name="wTf_sb")
            nc.sync.dma_start(wTf_sb[:], wTf.ap().rearrange("(k p) m -> p k m", p=P))
            bff_sb = const.tile([O, 1], F32, name="bff_sb")
            nc.sync.dma_start(bff_sb[:], bff.ap())
            rsA_sb = const.tile([O, R], F32, name="rsA_sb")
            nc.sync.dma_start(rsA_sb[:], rsA.ap())
            rsA2_sb = const.tile([O, R], F32, name="rsA2_sb")
            nc.sync.dma_start(rsA2_sb[:], rsA2.ap())
            disr1_sb = const.tile([O, R], F32, name="disr1_sb")
            nc.sync.dma_start(disr1_sb[:], disr1.ap())
            disr2_sb = const.tile([O, R], F32, name="disr2_sb")
            nc.sync.dma_start(disr2_sb[:], disr2.ap())
            dis1L_sb = const.tile([P, RT], F32, name="dis1L_sb")
            nc.sync.dma_start(dis1L_sb[:], dis1L.ap())
            dis2L_sb = const.tile([P, RT], F32, name="dis2L_sb")
            nc.sync.dma_start(dis2L_sb[:], dis2L.ap())

            # --- phase D: conv1, zT = [A@h; A2@h].T (raw), one adjacency
            # half at a time; BN partial stats issue right after each half
            zT_sb = feat.tile([P, H2M, R], F16, name="zT_sb")
            stat_sb = tmp.tile([P, 2 * H2M], F32, name="stat_sb", bufs=1)
            for half, src in ((0, adjT), (1, adjT2)):
                zps = {}
                for m in range(HM):
                    for ci in range(NC2):
                        zps[(m, ci)] = ps.tile(
                            [P, 512], F32, name=f"zps_{half}_{m}_{ci}",
                            tag=f"acc{(half * 4 + m * NC2 + ci) % 8}")
                for k in range(KT):
                    at = stream.tile([P, R], F16, name=f"c1_{half}_{k}", tag="adj", bufs=7)
                    nc.sync.dma_start(at[:], src[k * P:(k + 1) * P, :])
                    for m in range(HM):
                        for ci, (cs, cw) in enumerate(NCH):
                            nc.tensor.matmul(
                                zps[(m, ci)][:, :cw],
                                lhsT=hfull_sb[:, k, m * P:(m + 1) * P],
                                rhs=at[:, cs:cs + cw],
                                start=(k == 0), stop=(k == KT - 1),
                            )
                for m in range(HM):
                    for ci, (cs, cw) in enumerate(NCH):
                        nc.vector.tensor_copy(
                            out=zT_sb[:, half * HM + m, cs:cs + cw],
                            in_=zps[(m, ci)][:, :cw])
                # BN partial stats for this half's features: sums on vector,
                # square-sums on scalar (concurrent engines)
                for m in range(HM):
                    f = half * HM + m
                    nc.vector.tensor_reduce(
                        out=stat_sb[:, f:f + 1], in_=zT_sb[:, f, :],
                        axis=mybir.AxisListType.X, op=mybir.AluOpType.add)
                    sq2 = tmp.tile([P, R], F16, name="sq2", tag="sq", bufs=2)
                    nc.scalar.activation(
                        sq2[:], zT_sb[:, f, :],
                        mybir.ActivationFunctionType.Square,
                        accum_out=stat_sb[:, H2M + f:H2M + f + 1])

            # --- phase E: stats AllReduce + BN coefficients -------------
            ar_in = dram.tile([P, 2 * H2M], F32, name="ar_in")
            nc.gpsimd.dma_start(ar_in[:], stat_sb[:])
            ar_out = dram.tile([P, 2 * H2M], F32, name="ar_out")
            nc.gpsimd.collective_compute(
                "AllReduce", mybir.AluOpType.add, replica_groups=rg,
                ins=[ar_in.opt()], outs=[ar_out.opt()],
            )
            stat_g = tmp.tile([P, 2 * H2M], F32, name="stat_g", bufs=1)
            nc.gpsimd.dma_start(stat_g[:], ar_out[:])

            # fill the AllReduce window: local hT embed + sqrt-table preload
            # + the h-block start of the final-projection PSUM groups
            hT_sb = feat.tile([P, HM, R], F16, name="hT_sb")
            for m in range(HM):
                for ci, (cs, cw) in enumerate(NCH):
                    eps_t = ps.tile([P, 512], F32, name=f"eps_{m}_{ci}",
                                    tag=f"acc{6 + (m * NC2 + ci) % 2}")
                    for t in range(INK):
                        nc.tensor.matmul(
                            eps_t[:, :cw],
                            lhsT=wTe_sb[:, t, m * P:(m + 1) * P],
                            rhs=xT_sb[:, t, cs:cs + cw],
                            start=(t == 0), stop=(t == INK - 1),
                        )
                    nc.scalar.activation(
                        hT_sb[:, m, cs:cs + cw], eps_t[:, :cw],
                        mybir.ActivationFunctionType.Relu,
                        bias=be_sb[:, m:m + 1],
                    )
            eps_sb = tmp.tile([P, 1], F32, name="eps_sb", bufs=1)
            nc.vector.memset(eps_sb[:], BN_EPS)
            warm = tmp.tile([P, 1], F32, name="warm", bufs=1)
            nc.scalar.activation(
                warm[:], eps_sb[:], mybir.ActivationFunctionType.Sqrt,
                bias=eps_sb[:])
            fin = {}
            q1 = {}
            q2 = {}
            for ci in range(NC2):
                fin[ci] = ps.tile([O, 512], F32, name=f"fin_{ci}", tag=f"acc{ci}")
                q1[ci] = ps.tile([O, 512], F32, name=f"q1_{ci}", tag=f"acc{2 + ci}")
                q2[ci] = ps.tile([O, 512], F32, name=f"q2_{ci}", tag=f"acc{4 + ci}")
            for ci, (cs, cw) in enumerate(NCH):
                for t in range(HM):
                    nc.tensor.matmul(
                        fin[ci][:, :cw], lhsT=wTf_sb[:, t, :],
                        rhs=hT_sb[:, t, cs:cs + cw],
                        start=(t == 0), stop=False)

            # BN coefficients c, d
            cmean = tmp.tile([P, H2M], F32, name="cmean", bufs=1)
            nc.scalar.mul(cmean[:], stat_g[:, 0:H2M], 1.0 / NT)
            cvar = tmp.tile([P, H2M], F32, name="cvar", bufs=1)
            nc.scalar.mul(cvar[:], stat_g[:, H2M:2 * H2M], 1.0 / NT)
            msq = tmp.tile([P, H2M], F32, name="msq", bufs=1)
            nc.vector.tensor_mul(out=msq[:], in0=cmean[:], in1=cmean[:])
            nc.vector.tensor_tensor(
                out=cvar[:], in0=cvar[:], in1=msq[:],
                op=mybir.AluOpType.subtract)
            cstd = tmp.tile([P, H2M], F32, name="cstd", bufs=1)
            nc.scalar.activation(
                cstd[:], cvar[:], mybir.ActivationFunctionType.Sqrt,
                bias=eps_sb[:])
            crstd = tmp.tile([P, H2M], F32, name="crstd", bufs=1)
            nc.vector.reciprocal(crstd[:], cstd[:])
            c_t = tmp.tile([P, H2M], F32, name="c_t", bufs=1)
            nc.vector.tensor_mul(out=c_t[:], in0=crstd[:], in1=gam_sb[:])
            d_t = tmp.tile([P, H2M], F32, name="d_t", bufs=1)
            nc.vector.tensor_mul(out=d_t[:], in0=cmean[:], in1=c_t[:])
            nc.vector.tensor_tensor(
                out=d_t[:], in0=bet_sb[:], in1=d_t[:],
                op=mybir.AluOpType.subtract)
            d16 = tmp.tile([P, H2M], F16, name="d16", bufs=1)
            nc.vector.tensor_copy(out=d16[:], in_=d_t[:])

            # scale zT by c in place: zT <- c * z_raw (= z_n minus the d part)
            for f in range(H2M):
                nc.vector.tensor_scalar_mul(
                    zT_sb[:, f, :], zT_sb[:, f, :], c_t[:, f:f + 1])

            # --- phase G: pre-projections p1 = (cz)@W1.T, p2 = (cz)@W2.T
            pT_sb = tmp.tile([P, R], F16, name="pT_sb", bufs=1)
            for ci, (cs, cw) in enumerate(NCH):
                for j, base in enumerate((HM + H2M, HM + 2 * H2M)):
                    pps = ps.tile([O, 512], F32, name=f"pps_{j}_{ci}",
                                  tag=f"acc{6 + j}")
                    for t in range(H2M):
                        nc.tensor.matmul(
                            pps[:, :cw], lhsT=wTf_sb[:, base + t, :],
                            rhs=zT_sb[:, t, cs:cs + cw],
                            start=(t == 0), stop=(t == H2M - 1))
                    nc.vector.tensor_copy(
                        out=pT_sb[j * O:(j + 1) * O, cs:cs + cw],
                        in_=pps[:, :cw])

            # transpose p to node-major, scale rows by dis, gather in NGC
            # chunks so conv2' starts on the first chunk
            pcat_nm = tmp.tile([P, RT, P], F16, name="pcat_nm", bufs=1)
            pg_sb = feat.tile([P, NCORES, RT, P], F16, name="pg_sb")
            pg_chunks = []
            for gc in range(NGC):
                for nt in range(gc * RTC, (gc + 1) * RTC):
                    tps = ps.tile([P, P], F16, name=f"ptp_{nt}",
                                  tag=f"acc{6 + nt % 2}")
                    nc.tensor.transpose(
                        tps[:], pT_sb[:, nt * P:(nt + 1) * P], id16[:])
                    nc.vector.tensor_scalar_mul(
                        pcat_nm[:, nt, 0:O], tps[:, 0:O], dis1L_sb[:, nt:nt + 1])
                    nc.vector.tensor_scalar_mul(
                        pcat_nm[:, nt, O:P], tps[:, O:P], dis2L_sb[:, nt:nt + 1])
                pg_in = dram.tile([RTC * P, P], F16, name=f"pg_in_{gc}")
                nc.gpsimd.dma_start(
                    pg_in.rearrange("(nt p) f -> p nt f", p=P),
                    pcat_nm[:, gc * RTC:(gc + 1) * RTC, :])
                pg_o = dram.tile([NCORES, RTC * P, P], F16, name=f"pg_out_{gc}",
                                 addr_space="Shared")
                nc.gpsimd.collective_compute(
                    "AllGather", mybir.AluOpType.bypass, replica_groups=rg,
                    ins=[pg_in.opt()], outs=[pg_o.opt()],
                )
                pg_chunks.append(pg_o)
                for cr in range(NCORES):
                    nc.gpsimd.dma_start(
                        pg_sb[:, cr, gc * RTC:(gc + 1) * RTC, :],
                        pg_o[cr].rearrange("(nt p) f -> p nt f", p=P))

            # fill the AllGather window: s vectors, rank-1 prep, z-block
            # final matmuls (zT now c-scaled, wTf stays UNSCALED)
            s_cols = tmp.tile([O, 3], F32, name="s_cols", bufs=1)
            for j, base in enumerate((HM, HM + H2M, HM + 2 * H2M)):
                sps = ps.tile([O, 1], F32, name=f"sps_{j}", tag="acc6")
                for t in range(H2M):
                    nc.tensor.matmul(
                        sps[:], lhsT=wTf_sb[:, base + t, :],
                        rhs=d16[:, t:t + 1],
                        start=(t == 0), stop=(t == H2M - 1))
                nc.vector.tensor_copy(out=s_cols[:, j:j + 1], in_=sps[:])
            s0b = tmp.tile([O, 1], F32, name="s0b", bufs=1)
            nc.vector.tensor_add(out=s0b[:], in0=s_cols[:, 0:1], in1=bff_sb[:])
            # rkc = s0 + bias + s1*rsA + s2*rsA2  (shared across both ci)
            rkc = tmp.tile([O, R], F32, name="rkc", bufs=1)
            nc.vector.tensor_scalar_mul(rkc[:], rsA_sb[:], s_cols[:, 1:2])
            rk2 = tmp.tile([O, R], F32, name="rk2", bufs=1)
            nc.vector.tensor_scalar_mul(rk2[:], rsA2_sb[:], s_cols[:, 2:3])
            nc.vector.tensor_add(out=rkc[:], in0=rkc[:], in1=rk2[:])
            nc.vector.tensor_scalar_add(rkc[:], rkc[:], s0b[:])
            # z-block final matmuls (continue the fin PSUM groups)
            for ci, (cs, cw) in enumerate(NCH):
                for t in range(H2M):
                    nc.tensor.matmul(
                        fin[ci][:, :cw], lhsT=wTf_sb[:, HM + t, :],
                        rhs=zT_sb[:, t, cs:cs + cw],
                        start=False, stop=(t == H2M - 1))

            # --- phase H: conv2' binary SpMM (k ordered by gather chunk) -
            for gc in range(NGC):
                for cr in range(NCORES):
                    for nt in range(gc * RTC, (gc + 1) * RTC):
                        k = cr * RT + nt
                        first = (gc == 0 and cr == 0 and nt == 0)
                        last = (gc == NGC - 1 and cr == NCORES - 1
                                and nt == (gc + 1) * RTC - 1)
                        bt1 = stream.tile([P, R], BDT, name=f"b1_{k}",
                                          tag="adj8", bufs=13)
                        nc.sync.dma_start(bt1[:], Bp1[k * P:(k + 1) * P, :])
                        bt2 = stream.tile([P, R], BDT, name=f"b2_{k}",
                                          tag="adj8", bufs=13)
                        nc.sync.dma_start(bt2[:], Bp2[k * P:(k + 1) * P, :])
                        for ci, (cs, cw) in enumerate(NCH):
                            nc.tensor.matmul(
                                q1[ci][:, :cw], lhsT=pg_sb[:, cr, nt, 0:O],
                                rhs=bt1[:, cs:cs + cw],
                                start=first, stop=last)
                        for ci, (cs, cw) in enumerate(NCH):
                            nc.tensor.matmul(
                                q2[ci][:, :cw], lhsT=pg_sb[:, cr, nt, O:P],
                                rhs=bt2[:, cs:cs + cw],
                                start=first, stop=last)

            # combine: out = fin + rkc + dis1_i*q1 + dis2_i*q2; per-ci
            # transpose to node-major and write out incrementally
            outsb = tmp.tile([O, R], F32, name="outsb", bufs=1)
            o_nm = tmp.tile([P, RT, O], F32, name="o_nm", bufs=1)
            out_t = out.ap().rearrange("(nt p) o -> p nt o", p=P)
            for ci, (cs, cw) in enumerate(NCH):
                nc.vector.tensor_add(
                    out=outsb[:, cs:cs + cw], in0=fin[ci][:, :cw],
                    in1=rkc[:, cs:cs + cw])
                t1 = tmp.tile([O, 512], F32, name=f"t1_{ci}", tag="cmb", bufs=2)
                nc.vector.tensor_mul(
                    out=t1[:, :cw], in0=q1[ci][:, :cw],
                    in1=disr1_sb[:, cs:cs + cw])
                nc.vector.tensor_add(
                    out=outsb[:, cs:cs + cw], in0=outsb[:, cs:cs + cw],
                    in1=t1[:, :cw])
                t2 = tmp.tile([O, 512], F32, name=f"t2_{ci}", tag="cmb", bufs=2)
                nc.vector.tensor_mul(
                    out=t2[:, :cw], in0=q2[ci][:, :cw],
                    in1=disr2_sb[:, cs:cs + cw])
                nc.vector.tensor_add(
                    out=outsb[:, cs:cs + cw], in0=outsb[:, cs:cs + cw],
                    in1=t2[:, :cw])
                for nt in range(ci * RT // NC2, (ci + 1) * RT // NC2):
                    tps32 = ps.tile([P, O], F32, name=f"otp_{nt}",
                                    tag=f"acc{6 + nt % 2}")
                    nc.tensor.transpose(
                        tps32[:], outsb[:, nt * P:(nt + 1) * P], id32[:O, :O])
                    nc.any.tensor_copy(out=o_nm[:, nt, :], in_=tps32[:])
                nc.sync.dma_start(
                    out_t[:, ci * RT // NC2:(ci + 1) * RT // NC2, :],
                    o_nm[:, ci * RT // NC2:(ci + 1) * RT // NC2, :])

    nc.compile()
    return nc


_PROGRAM_CACHE = {}


def _get_program(NT, R):
    key = (NT, R)
    if key not in _PROGRAM_CACHE:
        _PROGRAM_CACHE[key] = build_program(NT, R)
    return _PROGRAM_CACHE[key]


def make_in_maps(inputs, NT, R):
    """Shard full inputs into per-core input maps (host-side, numpy)."""
    RT = R // P
    x = np.asarray(inputs["x"], np.float32)
    adj = np.asarray(inputs["adj_t"], np.float32)
    adj2 = np.asarray(inputs["adj_t2"], np.float32)
    we = np.asarray(inputs["w_embed"], np.float32)
    be = np.asarray(inputs["b_embed"], np.float32)
    gam = np.asarray(inputs["bn_gamma"], np.float32)
    bet = np.asarray(inputs["bn_beta"], np.float32)
    wf = np.asarray(inputs["w_fin"], np.float32)
    bf = np.asarray(inputs["b_fin"], np.float32)

    H2M = H2 // P
    xTf_h = np.ascontiguousarray(x.T).astype(np.float16)
    wTe_h = np.ascontiguousarray(we.T).astype(np.float16)
    be_h = np.ascontiguousarray(be.reshape(H // P, P).T).astype(np.float32)
    bebc_h = np.ascontiguousarray(
        np.broadcast_to(be[None, :], (P, H))).astype(np.float32)
    wTf_h = np.ascontiguousarray(wf.T).astype(np.float16)
    bff_h = np.ascontiguousarray(bf[:, None]).astype(np.float32)
    gam_h = np.ascontiguousarray(gam.reshape(H2M, P).T).astype(np.float32)
    bet_h = np.ascontiguousarray(bet.reshape(H2M, P).T).astype(np.float32)

    # binary decomposition of the normalized adjacencies
    B1 = adj > 0
    B2 = adj2 > 0
    d1 = B1.sum(1).astype(np.float32)
    d2 = B2.sum(1).astype(np.float32)
    dis1 = np.where(d1 > 0, 1.0 / np.sqrt(np.maximum(d1, 1e-12)), 0.0
                    ).astype(np.float32)
    dis2 = np.where(d2 > 0, 1.0 / np.sqrt(np.maximum(d2, 1e-12)), 0.0
                    ).astype(np.float32)
    bdt = ml_dtypes.float8_e4m3 if B_FP8 else np.float16

    in_maps = []
    for r in range(NCORES):
        rows = slice(r * R, (r + 1) * R)
        rsA_h = np.ascontiguousarray(
            np.broadcast_to(adj[rows].sum(1)[None, :], (O, R))).astype(np.float32)
        rsA2_h = np.ascontiguousarray(
            np.broadcast_to(adj2[rows].sum(1)[None, :], (O, R))).astype(np.float32)
        disr1_h = np.ascontiguousarray(
            np.broadcast_to(dis1[rows][None, :], (O, R))).astype(np.float32)
        disr2_h = np.ascontiguousarray(
            np.broadcast_to(dis2[rows][None, :], (O, R))).astype(np.float32)
        in_maps.append({
            "xTf": xTf_h,
            "xT": np.ascontiguousarray(x[rows].T).astype(np.float16),
            "adjT": np.ascontiguousarray(adj[rows].T).astype(np.float16),
            "adjT2": np.ascontiguousarray(adj2[rows].T).astype(np.float16),
            "Bp1": np.ascontiguousarray(B1[rows].T).astype(bdt),
            "Bp2": np.ascontiguousarray(B2[rows].T).astype(bdt),
            "wTe": wTe_h, "be": be_h, "bebc": bebc_h, "wTf": wTf_h,
            "bff": bff_h, "gam": gam_h, "bet": bet_h,
            "rsA": rsA_h, "rsA2": rsA2_h,
            "dis1L": np.ascontiguousarray(
                dis1[rows].reshape(RT, P).T).astype(np.float32),
            "dis2L": np.ascontiguousarray(
                dis2[rows].reshape(RT, P).T).astype(np.float32),
            "disr1": disr1_h, "disr2": disr2_h,
        })
    return in_maps


def kernel(**inputs):
    NT, R = FULL_CFG["NT"], FULL_CFG["R"]
    nc = _get_program(NT, R)
    in_maps = make_in_maps(inputs, NT, R)
    res = run_bass_kernel_spmd(nc, in_maps, core_ids=list(range(NCORES)))
    out = np.concatenate(
        [res.results[r]["out"] for r in range(NCORES)], axis=0)
    return out.astype(np.float32)
